# revision 79
# baseline (speedup 1.0000x reference)
"""Local (sliding-window causal) attention kernel for Trainium2, 8 NeuronCores.

Reference computation (per batch b, head h):
  q = x @ Wq + bq ; k = x @ Wk + bk ; v = x @ Wv + bv   (split into 16 heads of 64)
  S = q k^T / 8, masked to the causal band  i-255 <= j <= i
  out = softmax(S) @ v

Sharding: B=2, H=16 -> each of 8 cores owns a 128-column slice of the QKV
projections (2 heads) for both batches. Inputs are replicated; weights are
column-sliced per core; no collectives.

Device-side scheme per core (timing is dominated by the PE stream in the
cost model, so everything is arranged to minimize streamed matmul rows):
  - x is shipped as fp8-e4m3 hi + lo (residual) pairs, x ~= x_hi + x_lo.
  - Q^T, K^T: fp8 DoubleRow matmuls (K-tiles of 256) from x_hi and
    w_hi = fp8(32 W): psum ~= 32 Q^T. Evicted to bf16 with the (x32) bias.
  - V: 3 fp8 DoubleRow products (x_hi w_hi + x_lo w_hi + x_hi w_lo) so V is
    accurate to ~bf16 despite fp8 operands; evicted to bf16 V' = [V32 | 1].
  - S^T[kb] = K^T[kb].T @ Q^T[:, window 384] in bf16 (scores x1024).
  - exp on ACT with scale 2^-13; band mask applied MULTIPLICATIVELY after
    exp: diag triangle on GpSimd, tail triangle on DVE (bf16 2x mode).
  - PV per query block: 6 bf16 matmuls accumulate [128, 2, 65] in one PSUM
    bank (col 64 per head = row sums); evicted bf16 and stored unnormalized.
  - Host divides by (32 * row_sum), adds bv, reassembles. Softmax rows sum
    to 1 so bv folds in exactly.
  - PE is kept continuously busy from t~0 with warm-up matmuls so the
    clock p-state ramps to max before real work arrives.
"""

import sys

import numpy as np

try:
    import concourse.bass as bass  # noqa: F401
except ImportError:
    sys.path.insert(0, "/opt/trn_rl_repo")

import concourse.bass as bass
import concourse.tile as tile
from concourse import bacc, mybir
from concourse.bass import ts
from concourse.bass_utils import run_bass_kernel_spmd

import ml_dtypes

P = 128
B, L, D = 2, 2048, 1024
NT = B * L            # 4096 tokens
KSUB = D // P         # 8 contraction subtiles of 128
NDG = KSUB // 2       # 4 DoubleRow groups of 256
CHUNK = 512           # projection chunk (tokens)
NCH = NT // CHUNK     # 8
NLB = NT // P         # 32 token blocks
NKB = L // P          # 16 key blocks per batch
QW = 384              # query window per key block
DH = 64               # head dim
NCORES = 8
HPC = 2               # heads per core
EXP_SCALE = 0.125 / 1024.0   # 1/8 head scale, 1/(32*32) fp8 weight/score scale

F32 = mybir.dt.float32
BF16 = mybir.dt.bfloat16
F8 = mybir.dt.float8e4
DR = mybir.MatmulPerfMode.DoubleRow

# warm-up matmul free sizes (keep PE busy/ramping until real work arrives)
WARM_PRE = [128] * 3 + [512] * 8 + [128] * 6   # before first projection
WARM_K = 3                          # bridge after K(c0) before attends
WARM_V = 3                          # bridge before V(c0) (waits on xlo0)


def build_program():
    nc = bacc.Bacc("TRN2", target_bir_lowering=False, debug=False,
                   num_devices=NCORES)

    xhi_d = nc.dram_tensor("xhi", [P, NCH, KSUB, CHUNK], F8,
                           kind="ExternalInput").ap()
    xlo_d = nc.dram_tensor("xlo", [P, NCH, KSUB, CHUNK], F8,
                           kind="ExternalInput").ap()
    wqk_d = nc.dram_tensor("wqk", [P, 2, KSUB, P], F8,
                           kind="ExternalInput").ap()
    wv_d = nc.dram_tensor("wv", [P, 2, KSUB, P], F8,
                          kind="ExternalInput").ap()
    msk_d = nc.dram_tensor("msk", [P, 2, HPC, P], BF16,
                           kind="ExternalInput").ap()
    bqk_d = nc.dram_tensor("bqk", [P, 2], F32, kind="ExternalInput").ap()
    out_d = nc.dram_tensor("out", [P, B, NKB, HPC, DH + 1], BF16,
                           kind="ExternalOutput").ap()

    with tile.TileContext(nc) as tc:
        with (
            tc.tile_pool(name="const", bufs=1) as const,
            tc.tile_pool(name="xtp", bufs=1) as xtp,
            tc.tile_pool(name="qkv", bufs=1) as qkv,
        ):
            warm_sb = const.tile([P, 512], BF16)
            nc.vector.memset(warm_sb[:, 0:P], 0.25)
            nc.vector.memset(warm_sb[:, P:], 0.25)

            wqk_sb = const.tile([P, 2, KSUB, P], F8)
            wv_sb = const.tile([P, 2, KSUB, P], F8)
            msk_sb = const.tile([P, 2, HPC, P], BF16)
            bqk_sb = const.tile([P, 2], F32)
            xhi, xlo = [], []
            for c in range(NCH):
                th = xtp.tile([P, KSUB, CHUNK], F8, tag=f"xh{c}")
                tl = xtp.tile([P, KSUB, CHUNK], F8, tag=f"xl{c}")
                xhi.append(th)
                xlo.append(tl)
            # DMA issue order controls DMA_ENGINES transfer order: the
            # first-Q critical prefix [wqk, xhi0] leads on SP; small consts
            # ride the ACT queue.
            nc.sync.dma_start(wqk_sb[:], wqk_d)
            nc.sync.dma_start(xhi[0][:, 0:4, :], xhi_d[:, 0, 0:4, :])
            nc.sync.dma_start(xhi[0][:, 4:8, :], xhi_d[:, 0, 4:8, :])
            nc.sync.dma_start(wv_sb[:], wv_d)
            nc.sync.dma_start(xhi[1][:], xhi_d[:, 1])
            nc.sync.dma_start(xlo[0][:, 0:4, :], xlo_d[:, 0, 0:4, :])
            nc.sync.dma_start(xlo[0][:, 4:8, :], xlo_d[:, 0, 4:8, :])
            nc.scalar.dma_start(msk_sb[:], msk_d)
            nc.scalar.dma_start(bqk_sb[:], bqk_d)
            for c in range(2, NCH):
                nc.sync.dma_start(xhi[c][:], xhi_d[:, c])
                nc.sync.dma_start(xlo[c - 1][:], xlo_d[:, c - 1])
            nc.sync.dma_start(xlo[NCH - 1][:], xlo_d[:, NCH - 1])

            qt_sb = qkv.tile([P, NT], BF16, tag="qt")   # 32*Q^T, 2 heads
            kt_sb = qkv.tile([P, NT], BF16, tag="kt")   # 32*K^T
            v_sb = qkv.tile([P, HPC, NLB, DH + 1], BF16, tag="v")
            nc.vector.memset(v_sb[:, :, :, DH:DH + 1], 1.0)
            osb = [qkv.tile([P, NKB, HPC, DH + 1], BF16, tag=f"osb{b}",
                            name=f"osb{b}")
                   for b in range(B)]

            with (
                tc.tile_pool(name="pjps", bufs=2, space="PSUM") as pj_ps,
                tc.tile_pool(name="vgps", bufs=1, space="PSUM") as v_ps,
                tc.tile_pool(name="stps", bufs=2, space="PSUM") as st_ps,
                tc.tile_pool(name="ops", bufs=1, space="PSUM") as o_ps,
                tc.tile_pool(name="ptp", bufs=12) as ptp,
            ):
                def warm(n, w=512):
                    # keeps the PE clock p-state ramped while waiting on DMA
                    for _ in range(n):
                        ps = pj_ps.tile([P, CHUNK], F32, tag="pj",
                                        name="warm")
                        nc.tensor.matmul(ps[:, :w], lhsT=warm_sb[:, 0:P],
                                         rhs=warm_sb[:, :w],
                                         start=True, stop=True
                                         ).annotate("warm")

                for w in WARM_PRE:
                    warm(1, w)

                pts = {}

                def attend_st(b, kb):
                    qw = min(QW, L - kb * P)
                    k0 = b * L + kb * P
                    st = st_ps.tile([P, HPC, 512], F32, tag="st", name="st")
                    for h in range(HPC):
                        hs = h * DH
                        nc.tensor.matmul(st[:, h, 0:qw],
                                         lhsT=kt_sb[hs:hs + DH, k0:k0 + P],
                                         rhs=qt_sb[hs:hs + DH, k0:k0 + qw],
                                         start=True, stop=True
                                         ).annotate(f"st{b}.{kb}")
                    pt = ptp.tile([P, HPC, QW], BF16, tag="pt",
                                  name=f"pt{b}_{kb}")
                    pts[(b, kb)] = pt
                    # exp, per-head at the batch tail so trailing PVs can
                    # start after h0. Band masks are multiplicative: diag
                    # triangle on DVE (on the next PV's critical path),
                    # tail triangle on GpSimd (needed 2 key-blocks later);
                    # the middle 128 cols are fully valid.
                    hparts = ([slice(0, 1), slice(1, 2)] if False
                              else [slice(0, HPC)])
                    for hsl in hparts:
                        nc.scalar.activation(
                            pt[:, hsl, 0:qw], st[:, hsl, 0:qw],
                            mybir.ActivationFunctionType.Exp,
                            scale=EXP_SCALE)
                        nc.vector.tensor_tensor(
                            pt[:, hsl, 0:P], pt[:, hsl, 0:P],
                            msk_sb[:, 0, hsl], op=mybir.AluOpType.mult)
                        if qw == QW:
                            nc.gpsimd.tensor_tensor(
                                pt[:, hsl, 2 * P:3 * P],
                                pt[:, hsl, 2 * P:3 * P],
                                msk_sb[:, 1, hsl], op=mybir.AluOpType.mult)

                o_grp = [None]

                def emit_pv(b, qb):
                    # query-block pairs share one PSUM bank; col 64 of each
                    # head accumulates the softmax row sums (V' ones col).
                    # The last pair is split into two stop groups so qb14
                    # can evict+store while qb15 still accumulates.
                    if qb % 2 == 0:
                        o_grp[0] = o_ps.tile([P, 2, HPC, DH + 1], F32,
                                             tag="o", name="og")
                    o = o_grp[0]
                    sl = qb % 2
                    split = qb >= NKB - 2
                    kbs = list(range(max(qb - 2, 0), qb + 1))
                    for h in range(HPC):
                        for i, kb in enumerate(kbs):
                            qoff = (qb - kb) * P
                            nc.tensor.matmul(
                                o[:, sl, h, :],
                                lhsT=pts[(b, kb)][:, h, qoff:qoff + P],
                                rhs=v_sb[:, h, b * NKB + kb, :],
                                start=(sl == 0 and h == 0 and i == 0),
                                stop=(h == HPC - 1 and i == len(kbs) - 1
                                      and (split or sl == 1)),
                                skip_group_check=True
                                ).annotate(f"pv{b}.{qb}")
                    if split:
                        nc.vector.tensor_copy(
                            osb[b][:, qb:qb + 1, :, :], o[:, sl:sl + 1])
                        nc.sync.dma_start(out_d[:, b, qb:qb + 1],
                                          osb[b][:, qb:qb + 1])
                    elif sl == 1:
                        nc.vector.tensor_copy(
                            osb[b][:, qb - 1:qb + 1, :, :], o[:])
                        if qb == 11:
                            nc.scalar.dma_start(out_d[:, b, 0:12],
                                                osb[b][:, 0:12])
                        elif qb == 13:
                            nc.scalar.dma_start(out_d[:, b, 12:14],
                                                osb[b][:, 12:14])
                    if qb >= 2:
                        pts.pop((b, qb - 2))

                ready = {0: [0, 1], 1: [2, 3, 4, 5], 2: [6, 7, 8, 9],
                         3: [10, 11, 12, 13, 14, 15]}
                next_pv = {0: 0, 1: 0}
                v_done = {0: -1, 1: -1}  # highest batch-local lb emitted

                def attend(b, kb):
                    # pv trails the attends by 2 so pt (exp+mask) is ready
                    # by the time the PE reaches the PV matmuls; a pv may
                    # not be emitted before its V blocks exist in program
                    # order (the scheduler only syncs writes already seen)
                    attend_st(b, kb)
                    while next_pv[b] < kb - 2 and next_pv[b] <= v_done[b]:
                        emit_pv(b, next_pv[b])
                        next_pv[b] += 1

                def drain(b):
                    while next_pv[b] < NKB:
                        emit_pv(b, next_pv[b])
                        next_pv[b] += 1

                vgs = {}

                def v_phase3(b, c):
                    # deferred xlo products close chunk c's V group, then
                    # the group is evicted to V' bf16
                    vg = vgs.pop(c)
                    for lo in range(4):
                        for g in range(NDG):
                            nc.tensor.matmul(
                                vg[:, ts(lo, P)],
                                lhsT=xlo[c][:, 2 * g:2 * g + 2, ts(lo, P)],
                                rhs=wv_sb[:, 0, 2 * g:2 * g + 2, :],
                                start=False,
                                stop=(lo == 3 and g == NDG - 1),
                                perf_mode=DR,
                                skip_group_check=True
                                ).annotate(f"v{c}.{lo}.2")
                    nc.vector.tensor_copy(
                        v_sb[:, :, 4 * c:4 * c + 4, 0:DH],
                        vg[:].rearrange("p (l h d) -> p h l d",
                                        l=4, h=HPC))
                    v_done[b] = 4 * (c % 4) + 3

                for b in range(B):
                    for cc in range(4):
                        c = b * 4 + cc
                        kbs = ready[cc]
                        # Q then K projection chunk (fp8 DoubleRow). The
                        # batch-tail chunk runs as two 256-token groups in
                        # one bank (start only clears once) with split
                        # evicts, so its attends/exps unlock earlier.
                        segs = [(0, CHUNK)]
                        for j, dst, bcol in ((0, qt_sb, 0), (1, kt_sb, 1)):
                            ps = pj_ps.tile([P, CHUNK], F32, tag="pj",
                                            name="pj")
                            for si, (s0, s1) in enumerate(segs):
                                for g in range(NDG):
                                    nc.tensor.matmul(
                                        ps[:, s0:s1],
                                        lhsT=wqk_sb[:, j, 2 * g:2 * g + 2,
                                                    :],
                                        rhs=xhi[c][:, 2 * g:2 * g + 2,
                                                  s0:s1],
                                        start=(si == 0 and g == 0),
                                        stop=(g == NDG - 1),
                                        perf_mode=DR,
                                        skip_group_check=True
                                        ).annotate(f"qk{c}.{j}")
                                nc.vector.tensor_scalar_add(
                                    dst[:, c * CHUNK + s0:c * CHUNK + s1],
                                    ps[:, s0:s1],
                                    bqk_sb[:, bcol:bcol + 1])
                        if c == 0:
                            # xlo0 lands late; attends 0/1 only need qt/kt
                            warm(WARM_K)
                            attend_st(b, 0)
                            attend_st(b, 1)
                            warm(WARM_V)
                        # finish the previous chunk's V group (deferred xlo
                        # products): its xlo chunk only becomes critical
                        # here, a full chunk after its xhi
                        if cc > 0:
                            v_phase3(b, c - 1)
                        if c == 4:
                            # b0's last V close + PVs run here, overlapped
                            # with b1's first projections
                            v_phase3(0, 3)
                            drain(0)
                        # V chunk phases 1+2 (x_hi products) into a fresh
                        # single-bank accumulation group; attends weave
                        # between phases
                        if b == 1 and cc == 3:
                            # all tail attends first (no pv flushes): the
                            # terminal ACT exp chain issues earliest; V7
                            # products then run under it
                            for kb in kbs:
                                attend_st(b, kb)
                        vg = v_ps.tile([P, CHUNK], F32, tag="vg", name="vg")
                        vgs[c] = vg
                        for idx, j in enumerate((0, 1)):
                            for lo in range(4):
                                for g in range(NDG):
                                    nc.tensor.matmul(
                                        vg[:, ts(lo, P)],
                                        lhsT=xhi[c][:, 2 * g:2 * g + 2,
                                                    ts(lo, P)],
                                        rhs=wv_sb[:, j, 2 * g:2 * g + 2, :],
                                        start=(lo == 0 and idx == 0
                                               and g == 0),
                                        stop=False,
                                        perf_mode=DR,
                                        skip_group_check=True
                                        ).annotate(f"v{c}.{lo}.{idx}")
                            if (c != 0 and idx < len(kbs)
                                    and not (b == 1 and cc == 3)):
                                attend(b, kbs[idx])
                        if c != 0:
                            rest = kbs[2:]
                            if cc == 3 and b == B - 1:
                                # attends already ran before the V phases
                                v_phase3(b, c)
                            else:
                                for kb in rest:
                                    attend(b, kb)
                drain(1)
    nc.finalize()
    return nc


_NC = None


def _get_nc():
    global _NC
    if _NC is None:
        _NC = build_program()
    return _NC


def _masks():
    p = np.arange(P)[:, None]
    q = np.arange(P)[None, :]
    diag = (q >= p).astype(np.float32)          # [keys p, queries q]
    tail = (p > q).astype(np.float32)           # queries q+256
    m = np.empty((P, 2, HPC, P), np.float32)
    m[:, 0, 0] = m[:, 0, 1] = diag
    m[:, 1, 0] = m[:, 1, 1] = tail
    return m.astype(ml_dtypes.bfloat16)



def _fp8_pair(a):
    hi = a.astype(ml_dtypes.float8_e4m3)
    lo = (a - hi.astype(np.float32)).astype(ml_dtypes.float8_e4m3)
    return hi, lo


def _prepare_in_maps(inputs):
    hs = np.asarray(inputs["hidden_states"], np.float32)
    Wq = np.asarray(inputs["Wq"], np.float32)
    Wk = np.asarray(inputs["Wk"], np.float32)
    Wv = np.asarray(inputs["Wv"], np.float32)
    bq = np.asarray(inputs["bq"], np.float32)
    bk = np.asarray(inputs["bk"], np.float32)

    xT = hs.reshape(NT, D).T                     # [D, NT]
    x_hi, x_lo = _fp8_pair(xT)

    def xlayout(a):
        # [P, NCH, KSUB, CHUNK]: a8[p, c, k, t] = a[k*128+p, c*512+t]
        return np.ascontiguousarray(
            a.reshape(KSUB, P, NCH, CHUNK).transpose(1, 2, 0, 3))

    xhi8 = xlayout(x_hi)
    xlo8 = xlayout(x_lo)
    msk = _masks()

    def wslice(W, c):
        # [KSUB, P, P]: w[k, p, m] = W[k*128+p, c*128+m], scaled x32
        return 32.0 * W[:, c * P:(c + 1) * P].reshape(KSUB, P, P)

    in_maps = []
    for c in range(NCORES):
        wq_hi, _ = _fp8_pair(wslice(Wq, c))
        wk_hi, _ = _fp8_pair(wslice(Wk, c))
        wv_hi, wv_lo = _fp8_pair(wslice(Wv, c))
        wqk = np.stack([wq_hi, wk_hi], 0)        # [2, KSUB, P, P]
        wv = np.stack([wv_hi, wv_lo], 0)
        bqk = 32.0 * np.stack([bq[c * P:(c + 1) * P],
                               bk[c * P:(c + 1) * P]], 1)
        in_maps.append({
            "xhi": xhi8,
            "xlo": xlo8,
            "wqk": np.ascontiguousarray(wqk.transpose(2, 0, 1, 3)),
            "wv": np.ascontiguousarray(wv.transpose(2, 0, 1, 3)),
            "msk": msk,
            "bqk": np.ascontiguousarray(bqk, dtype=np.float32),
        })
    return in_maps


def run(inputs, trace=False, **kwargs):
    nc = _get_nc()
    in_maps = _prepare_in_maps(inputs)
    res = run_bass_kernel_spmd(nc, in_maps, core_ids=list(range(NCORES)),
                               trace=trace, **kwargs)
    bv = np.asarray(inputs["bv"], np.float32)
    full = np.empty((B, L, D), np.float32)
    for c in range(NCORES):
        raw = np.asarray(res.results[c]["out"], dtype=np.float32)
        # raw[p, b, qb, h, :]: 0:64 = 32*sum(p~ v32)/1024?? cols, 64 = sum(p~)
        o = raw[..., 0:DH] / (32.0 * raw[..., DH:DH + 1])
        # -> [B, NKB, P, HPC, DH] -> [B, L, HPC*DH]
        o = o.transpose(1, 2, 0, 3, 4).reshape(B, L, HPC * DH)
        full[:, :, c * P:(c + 1) * P] = o
    full = full + bv[None, None, :]
    return full.astype(np.float32), res


def kernel(**inputs):
    out, _ = run(inputs, trace=False)
    return out


# revision 80
# speedup vs baseline: 1.0060x; 1.0060x over previous
"""Local (sliding-window causal) attention kernel for Trainium2, 8 NeuronCores.

Reference computation (per batch b, head h):
  q = x @ Wq + bq ; k = x @ Wk + bk ; v = x @ Wv + bv   (split into 16 heads of 64)
  S = q k^T / 8, masked to the causal band  i-255 <= j <= i
  out = softmax(S) @ v

Sharding: B=2, H=16 -> each of 8 cores owns a 128-column slice of the QKV
projections (2 heads) for both batches. Inputs are replicated; weights are
column-sliced per core; no collectives.

Device-side scheme per core (timing is dominated by the PE stream in the
cost model, so everything is arranged to minimize streamed matmul rows):
  - x is shipped as fp8-e4m3 hi + lo (residual) pairs, x ~= x_hi + x_lo.
  - Q^T, K^T: fp8 DoubleRow matmuls (K-tiles of 256) from x_hi and
    w_hi = fp8(32 W): psum ~= 32 Q^T. Evicted to bf16 with the (x32) bias.
  - V: 3 fp8 DoubleRow products (x_hi w_hi + x_lo w_hi + x_hi w_lo) so V is
    accurate to ~bf16 despite fp8 operands; evicted to bf16 V' = [V32 | 1].
  - S^T[kb] = K^T[kb].T @ Q^T[:, window 384] in bf16 (scores x1024).
  - exp on ACT with scale 2^-13; band mask applied MULTIPLICATIVELY after
    exp: diag triangle on GpSimd, tail triangle on DVE (bf16 2x mode).
  - PV per query block: 6 bf16 matmuls accumulate [128, 2, 65] in one PSUM
    bank (col 64 per head = row sums); evicted bf16 and stored unnormalized.
  - Host divides by (32 * row_sum), adds bv, reassembles. Softmax rows sum
    to 1 so bv folds in exactly.
  - PE is kept continuously busy from t~0 with warm-up matmuls so the
    clock p-state ramps to max before real work arrives.
"""

import sys

import numpy as np

try:
    import concourse.bass as bass  # noqa: F401
except ImportError:
    sys.path.insert(0, "/opt/trn_rl_repo")

import concourse.bass as bass
import concourse.tile as tile
from concourse import bacc, mybir
from concourse.bass import ts
from concourse.bass_utils import run_bass_kernel_spmd

import ml_dtypes

P = 128
B, L, D = 2, 2048, 1024
NT = B * L            # 4096 tokens
KSUB = D // P         # 8 contraction subtiles of 128
NDG = KSUB // 2       # 4 DoubleRow groups of 256
CHUNK = 512           # projection chunk (tokens)
NCH = NT // CHUNK     # 8
NLB = NT // P         # 32 token blocks
NKB = L // P          # 16 key blocks per batch
QW = 384              # query window per key block
DH = 64               # head dim
NCORES = 8
HPC = 2               # heads per core
EXP_SCALE = 0.125 / 1024.0   # 1/8 head scale, 1/(32*32) fp8 weight/score scale

F32 = mybir.dt.float32
BF16 = mybir.dt.bfloat16
F8 = mybir.dt.float8e4
DR = mybir.MatmulPerfMode.DoubleRow

# warm-up matmul free sizes (keep PE busy/ramping until real work arrives)
WARM_PRE = [128] * 3 + [512] * 8 + [128] * 6   # before first projection
WARM_K = 3                          # bridge after K(c0) before attends
WARM_V = 3                          # bridge before V(c0) (waits on xlo0)


def build_program():
    nc = bacc.Bacc("TRN2", target_bir_lowering=False, debug=False,
                   num_devices=NCORES)

    xhi_d = nc.dram_tensor("xhi", [P, NCH, KSUB, CHUNK], F8,
                           kind="ExternalInput").ap()
    xlo_d = nc.dram_tensor("xlo", [P, NCH, KSUB, CHUNK], F8,
                           kind="ExternalInput").ap()
    wqk_d = nc.dram_tensor("wqk", [P, 2, KSUB, P], F8,
                           kind="ExternalInput").ap()
    wv_d = nc.dram_tensor("wv", [P, 2, KSUB, P], F8,
                          kind="ExternalInput").ap()
    msk_d = nc.dram_tensor("msk", [P, 2, HPC, P], BF16,
                           kind="ExternalInput").ap()
    bqk_d = nc.dram_tensor("bqk", [P, 2], F32, kind="ExternalInput").ap()
    out_d = nc.dram_tensor("out", [P, B, NKB, HPC, DH + 1], BF16,
                           kind="ExternalOutput").ap()

    with tile.TileContext(nc) as tc:
        with (
            tc.tile_pool(name="const", bufs=1) as const,
            tc.tile_pool(name="xtp", bufs=1) as xtp,
            tc.tile_pool(name="qkv", bufs=1) as qkv,
        ):
            warm_sb = const.tile([P, 512], BF16)
            nc.vector.memset(warm_sb[:, 0:P], 0.25)
            nc.vector.memset(warm_sb[:, P:], 0.25)

            wqk_sb = const.tile([P, 2, KSUB, P], F8)
            wv_sb = const.tile([P, 2, KSUB, P], F8)
            msk_sb = const.tile([P, 2, HPC, P], BF16)
            bqk_sb = const.tile([P, 2], F32)
            xhi, xlo = [], []
            for c in range(NCH):
                th = xtp.tile([P, KSUB, CHUNK], F8, tag=f"xh{c}")
                tl = xtp.tile([P, KSUB, CHUNK], F8, tag=f"xl{c}")
                xhi.append(th)
                xlo.append(tl)
            # DMA issue order controls DMA_ENGINES transfer order: the
            # first-Q critical prefix [wqk, xhi0] leads on SP; small consts
            # ride the ACT queue.
            nc.sync.dma_start(wqk_sb[:], wqk_d)
            nc.sync.dma_start(xhi[0][:, 0:4, :], xhi_d[:, 0, 0:4, :])
            nc.sync.dma_start(xhi[0][:, 4:8, :], xhi_d[:, 0, 4:8, :])
            nc.sync.dma_start(wv_sb[:], wv_d)
            nc.sync.dma_start(xhi[1][:], xhi_d[:, 1])
            nc.sync.dma_start(xlo[0][:, 0:4, :], xlo_d[:, 0, 0:4, :])
            nc.sync.dma_start(xlo[0][:, 4:8, :], xlo_d[:, 0, 4:8, :])
            nc.scalar.dma_start(msk_sb[:], msk_d)
            nc.scalar.dma_start(bqk_sb[:], bqk_d)
            for c in range(2, NCH):
                nc.sync.dma_start(xhi[c][:], xhi_d[:, c])
                nc.sync.dma_start(xlo[c - 1][:], xlo_d[:, c - 1])
            nc.sync.dma_start(xlo[NCH - 1][:], xlo_d[:, NCH - 1])

            qt_sb = qkv.tile([P, NT], BF16, tag="qt")   # 32*Q^T, 2 heads
            kt_sb = qkv.tile([P, NT], BF16, tag="kt")   # 32*K^T
            v_sb = qkv.tile([P, HPC, NLB, DH + 1], BF16, tag="v")
            nc.vector.memset(v_sb[:, :, :, DH:DH + 1], 1.0)
            osb = [qkv.tile([P, NKB, HPC, DH + 1], BF16, tag=f"osb{b}",
                            name=f"osb{b}")
                   for b in range(B)]

            with (
                tc.tile_pool(name="pjps", bufs=2, space="PSUM") as pj_ps,
                tc.tile_pool(name="vgps", bufs=1, space="PSUM") as v_ps,
                tc.tile_pool(name="stps", bufs=2, space="PSUM") as st_ps,
                tc.tile_pool(name="ops", bufs=1, space="PSUM") as o_ps,
                tc.tile_pool(name="ptp", bufs=8) as ptp,
            ):
                def warm(n, w=512):
                    # keeps the PE clock p-state ramped while waiting on DMA
                    for _ in range(n):
                        ps = pj_ps.tile([P, CHUNK], F32, tag="pj",
                                        name="warm")
                        nc.tensor.matmul(ps[:, :w], lhsT=warm_sb[:, 0:P],
                                         rhs=warm_sb[:, :w],
                                         start=True, stop=True
                                         ).annotate("warm")

                for w in WARM_PRE:
                    warm(1, w)

                pts = {}

                def attend_st(b, kb):
                    qw = min(QW, L - kb * P)
                    k0 = b * L + kb * P
                    st = st_ps.tile([P, HPC, 512], F32, tag="st", name="st")
                    for h in range(HPC):
                        hs = h * DH
                        nc.tensor.matmul(st[:, h, 0:qw],
                                         lhsT=kt_sb[hs:hs + DH, k0:k0 + P],
                                         rhs=qt_sb[hs:hs + DH, k0:k0 + qw],
                                         start=True, stop=True
                                         ).annotate(f"st{b}.{kb}")
                    pt = ptp.tile([P, HPC, QW], BF16, tag="pt",
                                  name=f"pt{b}_{kb}")
                    pts[(b, kb)] = pt
                    # exp, per-head at the batch tail so trailing PVs can
                    # start after h0. Band masks are multiplicative: diag
                    # triangle on DVE (on the next PV's critical path),
                    # tail triangle on GpSimd (needed 2 key-blocks later);
                    # the middle 128 cols are fully valid.
                    hparts = ([slice(0, 1), slice(1, 2)] if False
                              else [slice(0, HPC)])
                    for hsl in hparts:
                        nc.scalar.activation(
                            pt[:, hsl, 0:qw], st[:, hsl, 0:qw],
                            mybir.ActivationFunctionType.Exp,
                            scale=EXP_SCALE)
                        nc.vector.tensor_tensor(
                            pt[:, hsl, 0:P], pt[:, hsl, 0:P],
                            msk_sb[:, 0, hsl], op=mybir.AluOpType.mult)
                        if qw == QW:
                            nc.gpsimd.tensor_tensor(
                                pt[:, hsl, 2 * P:3 * P],
                                pt[:, hsl, 2 * P:3 * P],
                                msk_sb[:, 1, hsl], op=mybir.AluOpType.mult)

                o_grp = [None]

                def emit_pv(b, qb):
                    # query-block pairs share one PSUM bank; col 64 of each
                    # head accumulates the softmax row sums (V' ones col).
                    # The last pair is split into two stop groups so qb14
                    # can evict+store while qb15 still accumulates.
                    if qb % 2 == 0:
                        o_grp[0] = o_ps.tile([P, 2, HPC, DH + 1], F32,
                                             tag="o", name="og")
                    o = o_grp[0]
                    sl = qb % 2
                    split = qb >= NKB - 2
                    kbs = list(range(max(qb - 2, 0), qb + 1))
                    for h in range(HPC):
                        for i, kb in enumerate(kbs):
                            qoff = (qb - kb) * P
                            nc.tensor.matmul(
                                o[:, sl, h, :],
                                lhsT=pts[(b, kb)][:, h, qoff:qoff + P],
                                rhs=v_sb[:, h, b * NKB + kb, :],
                                start=(sl == 0 and h == 0 and i == 0),
                                stop=(h == HPC - 1 and i == len(kbs) - 1
                                      and (split or sl == 1)),
                                skip_group_check=True
                                ).annotate(f"pv{b}.{qb}")
                    if split:
                        nc.vector.tensor_copy(
                            osb[b][:, qb:qb + 1, :, :], o[:, sl:sl + 1])
                        nc.sync.dma_start(out_d[:, b, qb:qb + 1],
                                          osb[b][:, qb:qb + 1])
                    elif sl == 1:
                        nc.vector.tensor_copy(
                            osb[b][:, qb - 1:qb + 1, :, :], o[:])
                        if qb == 11:
                            nc.scalar.dma_start(out_d[:, b, 0:12],
                                                osb[b][:, 0:12])
                        elif qb == 13:
                            nc.scalar.dma_start(out_d[:, b, 12:14],
                                                osb[b][:, 12:14])
                    if qb >= 2:
                        pts.pop((b, qb - 2))

                ready = {0: [0, 1], 1: [2, 3, 4, 5], 2: [6, 7, 8, 9],
                         3: [10, 11, 12, 13, 14, 15]}
                next_pv = {0: 0, 1: 0}
                v_done = {0: -1, 1: -1}  # highest batch-local lb emitted

                def attend(b, kb):
                    # pv trails the attends by 2 so pt (exp+mask) is ready
                    # by the time the PE reaches the PV matmuls; a pv may
                    # not be emitted before its V blocks exist in program
                    # order (the scheduler only syncs writes already seen)
                    attend_st(b, kb)
                    while next_pv[b] < kb - 2 and next_pv[b] <= v_done[b]:
                        emit_pv(b, next_pv[b])
                        next_pv[b] += 1

                def drain(b):
                    while next_pv[b] < NKB:
                        emit_pv(b, next_pv[b])
                        next_pv[b] += 1

                vgs = {}

                def v_phase3(b, c):
                    # deferred xlo products close chunk c's V group, then
                    # the group is evicted to V' bf16
                    vg = vgs.pop(c)
                    for lo in range(4):
                        for g in range(NDG):
                            nc.tensor.matmul(
                                vg[:, ts(lo, P)],
                                lhsT=xlo[c][:, 2 * g:2 * g + 2, ts(lo, P)],
                                rhs=wv_sb[:, 0, 2 * g:2 * g + 2, :],
                                start=False,
                                stop=(lo == 3 and g == NDG - 1),
                                perf_mode=DR,
                                skip_group_check=True
                                ).annotate(f"v{c}.{lo}.2")
                    nc.vector.tensor_copy(
                        v_sb[:, :, 4 * c:4 * c + 4, 0:DH],
                        vg[:].rearrange("p (l h d) -> p h l d",
                                        l=4, h=HPC))
                    v_done[b] = 4 * (c % 4) + 3

                for b in range(B):
                    for cc in range(4):
                        c = b * 4 + cc
                        kbs = ready[cc]
                        # Q then K projection chunk (fp8 DoubleRow). The
                        # batch-tail chunk runs as two 256-token groups in
                        # one bank (start only clears once) with split
                        # evicts, so its attends/exps unlock earlier.
                        segs = [(0, CHUNK)]
                        for j, dst, bcol in ((0, qt_sb, 0), (1, kt_sb, 1)):
                            ps = pj_ps.tile([P, CHUNK], F32, tag="pj",
                                            name="pj")
                            for si, (s0, s1) in enumerate(segs):
                                for g in range(NDG):
                                    nc.tensor.matmul(
                                        ps[:, s0:s1],
                                        lhsT=wqk_sb[:, j, 2 * g:2 * g + 2,
                                                    :],
                                        rhs=xhi[c][:, 2 * g:2 * g + 2,
                                                  s0:s1],
                                        start=(si == 0 and g == 0),
                                        stop=(g == NDG - 1),
                                        perf_mode=DR,
                                        skip_group_check=True
                                        ).annotate(f"qk{c}.{j}")
                                nc.vector.tensor_scalar_add(
                                    dst[:, c * CHUNK + s0:c * CHUNK + s1],
                                    ps[:, s0:s1],
                                    bqk_sb[:, bcol:bcol + 1])
                        if c == 0:
                            # xlo0 lands late; attends 0/1 only need qt/kt
                            warm(WARM_K)
                            attend_st(b, 0)
                            attend_st(b, 1)
                            warm(WARM_V)
                        # finish the previous chunk's V group (deferred xlo
                        # products): its xlo chunk only becomes critical
                        # here, a full chunk after its xhi
                        if cc > 0:
                            v_phase3(b, c - 1)
                        if c == 4:
                            # b0's last V close + PVs run here, overlapped
                            # with b1's first projections
                            v_phase3(0, 3)
                            drain(0)
                        # V chunk phases 1+2 (x_hi products) into a fresh
                        # single-bank accumulation group; attends weave
                        # between phases
                        vg = v_ps.tile([P, CHUNK], F32, tag="vg", name="vg")
                        vgs[c] = vg
                        for idx, j in enumerate((0, 1)):
                            for lo in range(4):
                                for g in range(NDG):
                                    nc.tensor.matmul(
                                        vg[:, ts(lo, P)],
                                        lhsT=xhi[c][:, 2 * g:2 * g + 2,
                                                    ts(lo, P)],
                                        rhs=wv_sb[:, j, 2 * g:2 * g + 2, :],
                                        start=(lo == 0 and idx == 0
                                               and g == 0),
                                        stop=False,
                                        perf_mode=DR,
                                        skip_group_check=True
                                        ).annotate(f"v{c}.{lo}.{idx}")
                            if c != 0 and idx < len(kbs):
                                attend(b, kbs[idx])
                        if c != 0:
                            rest = kbs[2:]
                            if cc == 3 and b == B - 1:
                                # no later block to defer the V close into;
                                # run it after ALL tail attends so the ACT
                                # exp chain is issued as early as possible
                                for kb in rest:
                                    attend(b, kb)
                                v_phase3(b, c)
                            else:
                                for kb in rest:
                                    attend(b, kb)
                drain(1)
    nc.finalize()
    return nc


_NC = None


def _get_nc():
    global _NC
    if _NC is None:
        _NC = build_program()
    return _NC


def _masks():
    p = np.arange(P)[:, None]
    q = np.arange(P)[None, :]
    diag = (q >= p).astype(np.float32)          # [keys p, queries q]
    tail = (p > q).astype(np.float32)           # queries q+256
    m = np.empty((P, 2, HPC, P), np.float32)
    m[:, 0, 0] = m[:, 0, 1] = diag
    m[:, 1, 0] = m[:, 1, 1] = tail
    return m.astype(ml_dtypes.bfloat16)



def _fp8_pair(a):
    hi = a.astype(ml_dtypes.float8_e4m3)
    lo = (a - hi.astype(np.float32)).astype(ml_dtypes.float8_e4m3)
    return hi, lo


def _prepare_in_maps(inputs):
    hs = np.asarray(inputs["hidden_states"], np.float32)
    Wq = np.asarray(inputs["Wq"], np.float32)
    Wk = np.asarray(inputs["Wk"], np.float32)
    Wv = np.asarray(inputs["Wv"], np.float32)
    bq = np.asarray(inputs["bq"], np.float32)
    bk = np.asarray(inputs["bk"], np.float32)

    xT = hs.reshape(NT, D).T                     # [D, NT]
    x_hi, x_lo = _fp8_pair(xT)

    def xlayout(a):
        # [P, NCH, KSUB, CHUNK]: a8[p, c, k, t] = a[k*128+p, c*512+t]
        return np.ascontiguousarray(
            a.reshape(KSUB, P, NCH, CHUNK).transpose(1, 2, 0, 3))

    xhi8 = xlayout(x_hi)
    xlo8 = xlayout(x_lo)
    msk = _masks()

    def wslice(W, c):
        # [KSUB, P, P]: w[k, p, m] = W[k*128+p, c*128+m], scaled x32
        return 32.0 * W[:, c * P:(c + 1) * P].reshape(KSUB, P, P)

    in_maps = []
    for c in range(NCORES):
        wq_hi, _ = _fp8_pair(wslice(Wq, c))
        wk_hi, _ = _fp8_pair(wslice(Wk, c))
        wv_hi, wv_lo = _fp8_pair(wslice(Wv, c))
        wqk = np.stack([wq_hi, wk_hi], 0)        # [2, KSUB, P, P]
        wv = np.stack([wv_hi, wv_lo], 0)
        bqk = 32.0 * np.stack([bq[c * P:(c + 1) * P],
                               bk[c * P:(c + 1) * P]], 1)
        in_maps.append({
            "xhi": xhi8,
            "xlo": xlo8,
            "wqk": np.ascontiguousarray(wqk.transpose(2, 0, 1, 3)),
            "wv": np.ascontiguousarray(wv.transpose(2, 0, 1, 3)),
            "msk": msk,
            "bqk": np.ascontiguousarray(bqk, dtype=np.float32),
        })
    return in_maps


def run(inputs, trace=False, **kwargs):
    nc = _get_nc()
    in_maps = _prepare_in_maps(inputs)
    res = run_bass_kernel_spmd(nc, in_maps, core_ids=list(range(NCORES)),
                               trace=trace, **kwargs)
    bv = np.asarray(inputs["bv"], np.float32)
    full = np.empty((B, L, D), np.float32)
    for c in range(NCORES):
        raw = np.asarray(res.results[c]["out"], dtype=np.float32)
        # raw[p, b, qb, h, :]: 0:64 = 32*sum(p~ v32)/1024?? cols, 64 = sum(p~)
        o = raw[..., 0:DH] / (32.0 * raw[..., DH:DH + 1])
        # -> [B, NKB, P, HPC, DH] -> [B, L, HPC*DH]
        o = o.transpose(1, 2, 0, 3, 4).reshape(B, L, HPC * DH)
        full[:, :, c * P:(c + 1) * P] = o
    full = full + bv[None, None, :]
    return full.astype(np.float32), res


def kernel(**inputs):
    out, _ = run(inputs, trace=False)
    return out


# revision 81
# speedup vs baseline: 1.0061x; 1.0001x over previous
"""Local (sliding-window causal) attention kernel for Trainium2, 8 NeuronCores.

Reference computation (per batch b, head h):
  q = x @ Wq + bq ; k = x @ Wk + bk ; v = x @ Wv + bv   (split into 16 heads of 64)
  S = q k^T / 8, masked to the causal band  i-255 <= j <= i
  out = softmax(S) @ v

Sharding: B=2, H=16 -> each of 8 cores owns a 128-column slice of the QKV
projections (2 heads) for both batches. Inputs are replicated; weights are
column-sliced per core; no collectives.

Device-side scheme per core (timing is dominated by the PE stream in the
cost model, so everything is arranged to minimize streamed matmul rows):
  - x is shipped as fp8-e4m3 hi + lo (residual) pairs, x ~= x_hi + x_lo.
  - Q^T, K^T: fp8 DoubleRow matmuls (K-tiles of 256) from x_hi and
    w_hi = fp8(32 W): psum ~= 32 Q^T. Evicted to bf16 with the (x32) bias.
  - V: 3 fp8 DoubleRow products (x_hi w_hi + x_lo w_hi + x_hi w_lo) so V is
    accurate to ~bf16 despite fp8 operands; evicted to bf16 V' = [V32 | 1].
  - S^T[kb] = K^T[kb].T @ Q^T[:, window 384] in bf16 (scores x1024).
  - exp on ACT with scale 2^-13; band mask applied MULTIPLICATIVELY after
    exp: diag triangle on GpSimd, tail triangle on DVE (bf16 2x mode).
  - PV per query block: 6 bf16 matmuls accumulate [128, 2, 65] in one PSUM
    bank (col 64 per head = row sums); evicted bf16 and stored unnormalized.
  - Host divides by (32 * row_sum), adds bv, reassembles. Softmax rows sum
    to 1 so bv folds in exactly.
  - PE is kept continuously busy from t~0 with warm-up matmuls so the
    clock p-state ramps to max before real work arrives.
"""

import sys

import numpy as np

try:
    import concourse.bass as bass  # noqa: F401
except ImportError:
    sys.path.insert(0, "/opt/trn_rl_repo")

import concourse.bass as bass
import concourse.tile as tile
from concourse import bacc, mybir
from concourse.bass import ts
from concourse.bass_utils import run_bass_kernel_spmd

import ml_dtypes

P = 128
B, L, D = 2, 2048, 1024
NT = B * L            # 4096 tokens
KSUB = D // P         # 8 contraction subtiles of 128
NDG = KSUB // 2       # 4 DoubleRow groups of 256
CHUNK = 512           # projection chunk (tokens)
NCH = NT // CHUNK     # 8
NLB = NT // P         # 32 token blocks
NKB = L // P          # 16 key blocks per batch
QW = 384              # query window per key block
DH = 64               # head dim
NCORES = 8
HPC = 2               # heads per core
EXP_SCALE = 0.125 / 1024.0   # 1/8 head scale, 1/(32*32) fp8 weight/score scale

F32 = mybir.dt.float32
BF16 = mybir.dt.bfloat16
F8 = mybir.dt.float8e4
DR = mybir.MatmulPerfMode.DoubleRow

# warm-up matmul free sizes (keep PE busy/ramping until real work arrives)
WARM_PRE = [128] * 3 + [512] * 8 + [128] * 6   # before first projection
WARM_K = 3                          # bridge after K(c0) before attends
WARM_V = 3                          # bridge before V(c0) (waits on xlo0)


def build_program():
    nc = bacc.Bacc("TRN2", target_bir_lowering=False, debug=False,
                   num_devices=NCORES)

    xhi_d = nc.dram_tensor("xhi", [P, NCH, KSUB, CHUNK], F8,
                           kind="ExternalInput").ap()
    xlo_d = nc.dram_tensor("xlo", [P, NCH, KSUB, CHUNK], F8,
                           kind="ExternalInput").ap()
    wqk_d = nc.dram_tensor("wqk", [P, 2, KSUB, P], F8,
                           kind="ExternalInput").ap()
    wv_d = nc.dram_tensor("wv", [P, 2, KSUB, P], F8,
                          kind="ExternalInput").ap()
    msk_d = nc.dram_tensor("msk", [P, 2, HPC, P], BF16,
                           kind="ExternalInput").ap()
    bqk_d = nc.dram_tensor("bqk", [P, 2], F32, kind="ExternalInput").ap()
    out_d = nc.dram_tensor("out", [P, B, NKB, HPC, DH + 1], BF16,
                           kind="ExternalOutput").ap()

    with tile.TileContext(nc) as tc:
        with (
            tc.tile_pool(name="const", bufs=1) as const,
            tc.tile_pool(name="xtp", bufs=1) as xtp,
            tc.tile_pool(name="qkv", bufs=1) as qkv,
        ):
            warm_sb = const.tile([P, 512], BF16)
            nc.vector.memset(warm_sb[:, 0:P], 0.25)
            nc.vector.memset(warm_sb[:, P:], 0.25)

            wqk_sb = const.tile([P, 2, KSUB, P], F8)
            wv_sb = const.tile([P, 2, KSUB, P], F8)
            msk_sb = const.tile([P, 2, HPC, P], BF16)
            bqk_sb = const.tile([P, 2], F32)
            xhi, xlo = [], []
            for c in range(NCH):
                th = xtp.tile([P, KSUB, CHUNK], F8, tag=f"xh{c}")
                tl = xtp.tile([P, KSUB, CHUNK], F8, tag=f"xl{c}")
                xhi.append(th)
                xlo.append(tl)
            # DMA issue order controls DMA_ENGINES transfer order: the
            # first-Q critical prefix [wqk, xhi0] leads on SP; small consts
            # ride the ACT queue.
            nc.sync.dma_start(wqk_sb[:], wqk_d)
            nc.sync.dma_start(xhi[0][:, 0:4, :], xhi_d[:, 0, 0:4, :])
            nc.sync.dma_start(xhi[0][:, 4:8, :], xhi_d[:, 0, 4:8, :])
            nc.sync.dma_start(wv_sb[:], wv_d)
            nc.sync.dma_start(xhi[1][:], xhi_d[:, 1])
            nc.sync.dma_start(xlo[0][:, 0:4, :], xlo_d[:, 0, 0:4, :])
            nc.sync.dma_start(xlo[0][:, 4:8, :], xlo_d[:, 0, 4:8, :])
            nc.scalar.dma_start(msk_sb[:], msk_d)
            nc.scalar.dma_start(bqk_sb[:], bqk_d)
            for c in range(2, NCH):
                nc.sync.dma_start(xhi[c][:], xhi_d[:, c])
                nc.sync.dma_start(xlo[c - 1][:], xlo_d[:, c - 1])
            nc.sync.dma_start(xlo[NCH - 1][:], xlo_d[:, NCH - 1])

            qt_sb = qkv.tile([P, NT], BF16, tag="qt")   # 32*Q^T, 2 heads
            kt_sb = qkv.tile([P, NT], BF16, tag="kt")   # 32*K^T
            v_sb = qkv.tile([P, HPC, NLB, DH + 1], BF16, tag="v")
            nc.vector.memset(v_sb[:, :, :, DH:DH + 1], 1.0)
            osb = [qkv.tile([P, NKB, HPC, DH + 1], BF16, tag=f"osb{b}",
                            name=f"osb{b}")
                   for b in range(B)]

            with (
                tc.tile_pool(name="pjps", bufs=2, space="PSUM") as pj_ps,
                tc.tile_pool(name="vgps", bufs=1, space="PSUM") as v_ps,
                tc.tile_pool(name="stps", bufs=2, space="PSUM") as st_ps,
                tc.tile_pool(name="ops", bufs=1, space="PSUM") as o_ps,
                tc.tile_pool(name="ptp", bufs=8) as ptp,
            ):
                def warm(n, w=512):
                    # keeps the PE clock p-state ramped while waiting on DMA
                    for _ in range(n):
                        ps = pj_ps.tile([P, CHUNK], F32, tag="pj",
                                        name="warm")
                        nc.tensor.matmul(ps[:, :w], lhsT=warm_sb[:, 0:P],
                                         rhs=warm_sb[:, :w],
                                         start=True, stop=True
                                         ).annotate("warm")

                for w in WARM_PRE:
                    warm(1, w)

                pts = {}

                def attend_st(b, kb):
                    qw = min(QW, L - kb * P)
                    k0 = b * L + kb * P
                    st = st_ps.tile([P, HPC, 512], F32, tag="st", name="st")
                    for h in range(HPC):
                        hs = h * DH
                        nc.tensor.matmul(st[:, h, 0:qw],
                                         lhsT=kt_sb[hs:hs + DH, k0:k0 + P],
                                         rhs=qt_sb[hs:hs + DH, k0:k0 + qw],
                                         start=True, stop=True
                                         ).annotate(f"st{b}.{kb}")
                    pt = ptp.tile([P, HPC, QW], BF16, tag="pt",
                                  name=f"pt{b}_{kb}")
                    pts[(b, kb)] = pt
                    # exp, per-head at the batch tail so trailing PVs can
                    # start after h0. Band masks are multiplicative: diag
                    # triangle on DVE (on the next PV's critical path),
                    # tail triangle on GpSimd (needed 2 key-blocks later);
                    # the middle 128 cols are fully valid.
                    hparts = ([slice(0, 1), slice(1, 2)] if False
                              else [slice(0, HPC)])
                    for hsl in hparts:
                        nc.scalar.activation(
                            pt[:, hsl, 0:qw], st[:, hsl, 0:qw],
                            mybir.ActivationFunctionType.Exp,
                            scale=EXP_SCALE)
                        nc.vector.tensor_tensor(
                            pt[:, hsl, 0:P], pt[:, hsl, 0:P],
                            msk_sb[:, 0, hsl], op=mybir.AluOpType.mult)
                        if qw == QW:
                            nc.gpsimd.tensor_tensor(
                                pt[:, hsl, 2 * P:3 * P],
                                pt[:, hsl, 2 * P:3 * P],
                                msk_sb[:, 1, hsl], op=mybir.AluOpType.mult)

                o_grp = [None]

                def emit_pv(b, qb):
                    # query-block pairs share one PSUM bank; col 64 of each
                    # head accumulates the softmax row sums (V' ones col).
                    # The last pair is split into two stop groups so qb14
                    # can evict+store while qb15 still accumulates.
                    if qb % 2 == 0:
                        o_grp[0] = o_ps.tile([P, 2, HPC, DH + 1], F32,
                                             tag="o", name="og")
                    o = o_grp[0]
                    sl = qb % 2
                    split = qb >= NKB - 2
                    kbs = list(range(max(qb - 2, 0), qb + 1))
                    for h in range(HPC):
                        for i, kb in enumerate(kbs):
                            qoff = (qb - kb) * P
                            nc.tensor.matmul(
                                o[:, sl, h, :],
                                lhsT=pts[(b, kb)][:, h, qoff:qoff + P],
                                rhs=v_sb[:, h, b * NKB + kb, :],
                                start=(sl == 0 and h == 0 and i == 0),
                                stop=(h == HPC - 1 and i == len(kbs) - 1
                                      and (split or sl == 1)),
                                skip_group_check=True
                                ).annotate(f"pv{b}.{qb}")
                    if split:
                        nc.vector.tensor_copy(
                            osb[b][:, qb:qb + 1, :, :], o[:, sl:sl + 1])
                        nc.sync.dma_start(out_d[:, b, qb:qb + 1],
                                          osb[b][:, qb:qb + 1])
                    elif sl == 1:
                        nc.vector.tensor_copy(
                            osb[b][:, qb - 1:qb + 1, :, :], o[:])
                        if qb == 11:
                            nc.scalar.dma_start(out_d[:, b, 0:12],
                                                osb[b][:, 0:12])
                        elif qb == 13:
                            nc.scalar.dma_start(out_d[:, b, 12:14],
                                                osb[b][:, 12:14])
                    if qb >= 2:
                        pts.pop((b, qb - 2))

                ready = {0: [0, 1], 1: [2, 3, 4, 5], 2: [6, 7, 8, 9],
                         3: [10, 11, 12, 13, 14, 15]}
                next_pv = {0: 0, 1: 0}
                v_done = {0: -1, 1: -1}  # highest batch-local lb emitted

                def attend(b, kb):
                    # pv trails the attends by 2 so pt (exp+mask) is ready
                    # by the time the PE reaches the PV matmuls; a pv may
                    # not be emitted before its V blocks exist in program
                    # order (the scheduler only syncs writes already seen)
                    attend_st(b, kb)
                    while next_pv[b] < kb - 2 and next_pv[b] <= v_done[b]:
                        emit_pv(b, next_pv[b])
                        next_pv[b] += 1

                def drain(b):
                    while next_pv[b] < NKB:
                        emit_pv(b, next_pv[b])
                        next_pv[b] += 1

                vgs = {}

                def v_phase3(b, c):
                    # deferred xlo products close chunk c's V group, then
                    # the group is evicted to V' bf16
                    vg = vgs.pop(c)
                    for lo in range(4):
                        for g in range(NDG):
                            nc.tensor.matmul(
                                vg[:, ts(lo, P)],
                                lhsT=xlo[c][:, 2 * g:2 * g + 2, ts(lo, P)],
                                rhs=wv_sb[:, 0, 2 * g:2 * g + 2, :],
                                start=False,
                                stop=(lo == 3 and g == NDG - 1),
                                perf_mode=DR,
                                skip_group_check=True
                                ).annotate(f"v{c}.{lo}.2")
                    nc.vector.tensor_copy(
                        v_sb[:, :, 4 * c:4 * c + 4, 0:DH],
                        vg[:].rearrange("p (l h d) -> p h l d",
                                        l=4, h=HPC))
                    v_done[b] = 4 * (c % 4) + 3

                for b in range(B):
                    for cc in range(4):
                        c = b * 4 + cc
                        kbs = ready[cc]
                        # Q then K projection chunk (fp8 DoubleRow). The
                        # batch-tail chunk runs as two 256-token groups in
                        # one bank (start only clears once) with split
                        # evicts, so its attends/exps unlock earlier.
                        segs = [(0, CHUNK)]
                        for j, dst, bcol in ((0, qt_sb, 0), (1, kt_sb, 1)):
                            ps = pj_ps.tile([P, CHUNK], F32, tag="pj",
                                            name="pj")
                            for si, (s0, s1) in enumerate(segs):
                                for g in range(NDG):
                                    nc.tensor.matmul(
                                        ps[:, s0:s1],
                                        lhsT=wqk_sb[:, j, 2 * g:2 * g + 2,
                                                    :],
                                        rhs=xhi[c][:, 2 * g:2 * g + 2,
                                                  s0:s1],
                                        start=(si == 0 and g == 0),
                                        stop=(g == NDG - 1),
                                        perf_mode=DR,
                                        skip_group_check=True
                                        ).annotate(f"qk{c}.{j}")
                                nc.vector.tensor_scalar_add(
                                    dst[:, c * CHUNK + s0:c * CHUNK + s1],
                                    ps[:, s0:s1],
                                    bqk_sb[:, bcol:bcol + 1])
                        if c == 0:
                            # xlo0 lands late; attends 0/1 only need qt/kt
                            warm(WARM_K)
                            attend_st(b, 0)
                            attend_st(b, 1)
                            warm(WARM_V)
                        # finish the previous chunk's V group (deferred xlo
                        # products): its xlo chunk only becomes critical
                        # here, a full chunk after its xhi
                        if cc > 0:
                            v_phase3(b, c - 1)
                        if c == 4:
                            # b0's V close must precede vg(4)'s bank reuse;
                            # its PV drain waits until after b1's first
                            # attends so their exps fill the idle ACT
                            v_phase3(0, 3)
                        # V chunk phases 1+2 (x_hi products) into a fresh
                        # single-bank accumulation group; attends weave
                        # between phases
                        vg = v_ps.tile([P, CHUNK], F32, tag="vg", name="vg")
                        vgs[c] = vg
                        for idx, j in enumerate((0, 1)):
                            for lo in range(4):
                                for g in range(NDG):
                                    nc.tensor.matmul(
                                        vg[:, ts(lo, P)],
                                        lhsT=xhi[c][:, 2 * g:2 * g + 2,
                                                    ts(lo, P)],
                                        rhs=wv_sb[:, j, 2 * g:2 * g + 2, :],
                                        start=(lo == 0 and idx == 0
                                               and g == 0),
                                        stop=False,
                                        perf_mode=DR,
                                        skip_group_check=True
                                        ).annotate(f"v{c}.{lo}.{idx}")
                            if c != 0 and idx < len(kbs):
                                attend(b, kbs[idx])
                        if c == 4:
                            drain(0)
                        if c != 0:
                            rest = kbs[2:]
                            if cc == 3 and b == B - 1:
                                # no later block to defer the V close into;
                                # run it after ALL tail attends so the ACT
                                # exp chain is issued as early as possible
                                for kb in rest:
                                    attend(b, kb)
                                v_phase3(b, c)
                            else:
                                for kb in rest:
                                    attend(b, kb)
                drain(1)
    nc.finalize()
    return nc


_NC = None


def _get_nc():
    global _NC
    if _NC is None:
        _NC = build_program()
    return _NC


def _masks():
    p = np.arange(P)[:, None]
    q = np.arange(P)[None, :]
    diag = (q >= p).astype(np.float32)          # [keys p, queries q]
    tail = (p > q).astype(np.float32)           # queries q+256
    m = np.empty((P, 2, HPC, P), np.float32)
    m[:, 0, 0] = m[:, 0, 1] = diag
    m[:, 1, 0] = m[:, 1, 1] = tail
    return m.astype(ml_dtypes.bfloat16)



def _fp8_pair(a):
    hi = a.astype(ml_dtypes.float8_e4m3)
    lo = (a - hi.astype(np.float32)).astype(ml_dtypes.float8_e4m3)
    return hi, lo


def _prepare_in_maps(inputs):
    hs = np.asarray(inputs["hidden_states"], np.float32)
    Wq = np.asarray(inputs["Wq"], np.float32)
    Wk = np.asarray(inputs["Wk"], np.float32)
    Wv = np.asarray(inputs["Wv"], np.float32)
    bq = np.asarray(inputs["bq"], np.float32)
    bk = np.asarray(inputs["bk"], np.float32)

    xT = hs.reshape(NT, D).T                     # [D, NT]
    x_hi, x_lo = _fp8_pair(xT)

    def xlayout(a):
        # [P, NCH, KSUB, CHUNK]: a8[p, c, k, t] = a[k*128+p, c*512+t]
        return np.ascontiguousarray(
            a.reshape(KSUB, P, NCH, CHUNK).transpose(1, 2, 0, 3))

    xhi8 = xlayout(x_hi)
    xlo8 = xlayout(x_lo)
    msk = _masks()

    def wslice(W, c):
        # [KSUB, P, P]: w[k, p, m] = W[k*128+p, c*128+m], scaled x32
        return 32.0 * W[:, c * P:(c + 1) * P].reshape(KSUB, P, P)

    in_maps = []
    for c in range(NCORES):
        wq_hi, _ = _fp8_pair(wslice(Wq, c))
        wk_hi, _ = _fp8_pair(wslice(Wk, c))
        wv_hi, wv_lo = _fp8_pair(wslice(Wv, c))
        wqk = np.stack([wq_hi, wk_hi], 0)        # [2, KSUB, P, P]
        wv = np.stack([wv_hi, wv_lo], 0)
        bqk = 32.0 * np.stack([bq[c * P:(c + 1) * P],
                               bk[c * P:(c + 1) * P]], 1)
        in_maps.append({
            "xhi": xhi8,
            "xlo": xlo8,
            "wqk": np.ascontiguousarray(wqk.transpose(2, 0, 1, 3)),
            "wv": np.ascontiguousarray(wv.transpose(2, 0, 1, 3)),
            "msk": msk,
            "bqk": np.ascontiguousarray(bqk, dtype=np.float32),
        })
    return in_maps


def run(inputs, trace=False, **kwargs):
    nc = _get_nc()
    in_maps = _prepare_in_maps(inputs)
    res = run_bass_kernel_spmd(nc, in_maps, core_ids=list(range(NCORES)),
                               trace=trace, **kwargs)
    bv = np.asarray(inputs["bv"], np.float32)
    full = np.empty((B, L, D), np.float32)
    for c in range(NCORES):
        raw = np.asarray(res.results[c]["out"], dtype=np.float32)
        # raw[p, b, qb, h, :]: 0:64 = 32*sum(p~ v32)/1024?? cols, 64 = sum(p~)
        o = raw[..., 0:DH] / (32.0 * raw[..., DH:DH + 1])
        # -> [B, NKB, P, HPC, DH] -> [B, L, HPC*DH]
        o = o.transpose(1, 2, 0, 3, 4).reshape(B, L, HPC * DH)
        full[:, :, c * P:(c + 1) * P] = o
    full = full + bv[None, None, :]
    return full.astype(np.float32), res


def kernel(**inputs):
    out, _ = run(inputs, trace=False)
    return out
